# revision 1
# baseline (speedup 1.0000x reference)
"""Distributed KNN (k-nearest-neighbor classify) on 8 Trainium2 NeuronCores.

Strategy (per sharding hint): shard X_train/y_train along num_train across the
8 cores. Each core computes its [1024, 12500] slab of adjusted scores
    s[t, n] = X[t] . Xtr[n] - 0.5*||Xtr[n]||^2
(monotonically equivalent to negative squared euclidean distance per test row)
via TensorE matmuls (K=128 feature contraction + K=1 PSUM-accumulate fold of
the -0.5*||t||^2 bias), then uses the DVE MAX8/MAX_INDEX sort hardware to pull
the top-8 (value, index) per 500-candidate tile. The 25*8=200 candidates per
test per core are DMA'd out; the host merges 8*200=1600 candidates/test,
takes the global top-k (value desc, index asc — matching jax.lax.top_k tie
semantics), gathers labels and majority-votes (argmax -> smallest label on
ties, matching the reference).
"""
import numpy as np
from contextlib import ExitStack

# Problem geometry (hardcoded per contract).
D = 128          # feature dim = contraction dim = partition dim
T = 1024         # num test points
N_TRAIN = 100000
N_CORES = 8
NS = N_TRAIN // N_CORES   # 12500 train points per core
TILE = 500                # candidates per matmul tile (one PSUM bank, <=512 fp32)
NT = NS // TILE           # 25 tiles per core
NG = T // 128             # 8 test groups of 128 (PSUM partition dim)
NCAND = NT * 8            # 200 candidates kept per test per core
NUM_CLASSES = 10

_CACHE = {}


def _build_program():
    import concourse.tile as tile
    from concourse import bacc, mybir

    F32 = mybir.dt.float32
    U32 = mybir.dt.uint32

    nc = bacc.Bacc("TRN2", target_bir_lowering=False, debug=False,
                   num_devices=N_CORES)
    xT = nc.dram_tensor("xT", [D, T], F32, kind="ExternalInput").ap()
    xtrT = nc.dram_tensor("xtrT", [D, NS], F32, kind="ExternalInput").ap()
    negq = nc.dram_tensor("negq", [1, NS], F32, kind="ExternalInput").ap()
    ones = nc.dram_tensor("ones", [1, D], F32, kind="ExternalInput").ap()
    out_vals = nc.dram_tensor("vals", [T, NCAND], F32, kind="ExternalOutput").ap()
    out_idx = nc.dram_tensor("idx", [T, NCAND], U32, kind="ExternalOutput").ap()

    with tile.TileContext(nc) as tc:
        with ExitStack() as ctx:
            consts = ctx.enter_context(tc.tile_pool(name="consts", bufs=1))
            xT_sb = consts.tile([D, T], F32, name="xT_sb", tag="xT")
            nc.sync.dma_start(xT_sb[:], xT[:])
            ones_sb = consts.tile([1, D], F32, name="ones_sb", tag="ones")
            nc.sync.dma_start(ones_sb[:], ones[:])
            negq_sb = consts.tile([1, NS], F32, name="negq_sb", tag="negq")
            nc.sync.dma_start(negq_sb[:], negq[:])

            xtr_pool = ctx.enter_context(tc.tile_pool(name="xtr", bufs=4))
            psum_pool = ctx.enter_context(
                tc.tile_pool(name="ps", bufs=8, space="PSUM"))
            cand = ctx.enter_context(tc.tile_pool(name="cand", bufs=1))
            vals_sb = [cand.tile([128, NCAND], F32, name=f"v{g}", tag=f"v{g}")
                       for g in range(NG)]
            idx_sb = [cand.tile([128, NCAND], U32, name=f"i{g}", tag=f"i{g}")
                      for g in range(NG)]

            for i in range(NT):
                xtr_t = xtr_pool.tile([D, TILE], F32, name="xtr_t")
                nc.sync.dma_start(xtr_t[:], xtrT[:, i * TILE:(i + 1) * TILE])
                for g in range(NG):
                    ps = psum_pool.tile([128, TILE], F32, name="ps")
                    nc.tensor.matmul(ps[:], xT_sb[:, g * 128:(g + 1) * 128],
                                     xtr_t[:], start=True, stop=False)
                    nc.tensor.matmul(ps[:], ones_sb[:1, :],
                                     negq_sb[:1, i * TILE:(i + 1) * TILE],
                                     start=False, stop=True)
                    vslice = vals_sb[g][:, i * 8:(i + 1) * 8]
                    nc.vector.max(vslice, ps[:])
                    nc.vector.max_index(idx_sb[g][:, i * 8:(i + 1) * 8],
                                        vslice, ps[:])
            for g in range(NG):
                nc.sync.dma_start(out_vals[g * 128:(g + 1) * 128, :], vals_sb[g][:])
                nc.sync.dma_start(out_idx[g * 128:(g + 1) * 128, :], idx_sb[g][:])
    nc.compile()
    return nc


def _get_program():
    if "nc" not in _CACHE:
        _CACHE["nc"] = _build_program()
    return _CACHE["nc"]


def _prep_in_maps(X, X_train):
    xT = np.ascontiguousarray(X.T)
    ones = np.ones((1, D), dtype=np.float32)
    in_maps = []
    for c in range(N_CORES):
        shard = X_train[c * NS:(c + 1) * NS]
        xtrT = np.ascontiguousarray(shard.T)
        q = np.einsum("nd,nd->n", shard.astype(np.float64),
                      shard.astype(np.float64))
        negq = (-0.5 * q).astype(np.float32)[None, :]
        in_maps.append({"xT": xT, "xtrT": xtrT, "negq": negq, "ones": ones})
    return in_maps


def _merge_and_vote(results, y_train, k):
    tile_off = np.repeat(np.arange(NT, dtype=np.int64) * TILE, 8)
    all_vals = np.empty((T, N_CORES * NCAND), dtype=np.float32)
    all_idx = np.empty((T, N_CORES * NCAND), dtype=np.int64)
    for c in range(N_CORES):
        vals = results[c]["vals"]
        idx = results[c]["idx"].astype(np.int64) + tile_off[None, :] + c * NS
        all_vals[:, c * NCAND:(c + 1) * NCAND] = vals
        all_idx[:, c * NCAND:(c + 1) * NCAND] = idx

    # top-k by (value desc, global index asc) — matches lax.top_k on -dists.
    order = np.lexsort((all_idx, -all_vals))[:, :k]
    idx_k = np.take_along_axis(all_idx, order, axis=1)
    labels = y_train[idx_k]                                   # [T, k]
    counts = (labels[:, :, None] == np.arange(NUM_CLASSES)).sum(axis=1)
    return np.argmax(counts, axis=1).astype(np.float32)


def kernel(X, X_train, y_train, k):
    from concourse.bass_utils import run_bass_kernel_spmd

    X = np.asarray(X, dtype=np.float32)
    X_train = np.asarray(X_train, dtype=np.float32)
    y_train = np.asarray(y_train)
    k = int(k)
    assert X.shape == (T, D) and X_train.shape == (N_TRAIN, D)
    assert 1 <= k <= 8

    nc = _get_program()
    in_maps = _prep_in_maps(X, X_train)
    res = run_bass_kernel_spmd(nc, in_maps, core_ids=list(range(N_CORES)))
    return _merge_and_vote(res.results, y_train, k)
